# revision 24
# baseline (speedup 1.0000x reference)
"""Causal single-head attention (B=4, S=2048, D=768) on 8 trn2 NeuronCores.

Sharding: batch (4) x query-split (2). Core c = 2*b + r handles batch b and
the 8 interleaved query blocks {2i+r : i=0..7} (128 rows each).

Algebraic restructuring vs the straightforward QKV pipeline:
  scores = (X Wq)(X Wk)^T = X G X^T          with G = Wq Wk^T (host-packed)
  out    = softmax(scores) X Wv              (values = raw X; Wv applied last)
so neither K nor V is ever materialized. Per-core matmul stream drops from
~298k to ~194k PE columns.

All inputs are host-permuted to partition-major [128, N] layouts so each
tensor lands in SBUF with 1-2 large DMA transfers (>=0.6 MB each; small
strided transfers measured ~7x below line rate).

Pipeline per core (matmuls as out = lhsT.T @ rhs, bf16 inputs):
  Phase T : TT[e, q] = G^T @ XTq             (T = Xq G; same cost as Q proj)
  Phase A : two passes over q-halves (slots 0-3, 4-7). Per key block j:
              ST_j[k, q-window] = X_j @ T^T  (xt chunks vs tt chunks)
              +mask on the window's first 128 cols (diag tril / pad kill),
              exp -> PT_j (SBUF bf16)
            then 7 accumulating matmuls into per-d-chunk PSUM banks:
              PXT[d-chunk][:, window] += X_j-chunk-stationary @ PT_j
            chunk 6 uses an all-ones stationary -> replicated row sums.
            Pass end: recip = approx(1/rowsum); pxt_sb[c] = PXT[c] * recip.
  Phase O : O[q-block] = sum_c pxt_sb[c]-chunk-stationary @ Wv[c] -> bf16 out.

Role asymmetry (which key block is diagonal / padded) is carried by the mask
INPUT so the same SPMD program runs on all 8 cores. PSUM: 7 accumulator
banks + 1 ST bank = exactly 8. Variable-width accumulation is legal because
key-block j=0 covers each bank's full 512 cols (start=True clears whole
bank) and later, narrower windows only accumulate (per-element has_written).
"""

import os
import sys

for _p in ("/opt/trn_rl_repo", "/root/.axon_site/_ro/trn_rl_repo"):
    if os.path.isdir(_p) and _p not in sys.path:
        sys.path.append(_p)

import numpy as np

import concourse.bacc as bacc
import concourse.mybir as mybir
import concourse.tile as tile
from concourse._compat import get_trn_type

B, S, D = 4, 2048, 768
P = 128
DC = D // P          # 6 contraction / d chunks
SB = S // P          # 16 seq blocks
NQ = 8               # q-slots per core
QW = NQ * P          # 1024 q rows per core
SCALE = 1.0 / float(np.sqrt(D))
MASK_VAL = -1e30

F32 = mybir.dt.float32
BF16 = mybir.dt.bfloat16
DT_IN = BF16


def build_nc(reps=1):
    nc = bacc.Bacc(
        get_trn_type() or "TRN2",
        target_bir_lowering=False,
        debug=False,
        num_devices=8,
        dynamic_dma_scratch_size=2048,
    )
    # all host-permuted to partition-major [128, N]; see build_in_maps.
    xt_d = nc.dram_tensor("xt", [P, SB * DC * P], DT_IN, kind="ExternalInput").ap()
    xtq_d = nc.dram_tensor("xtq", [P, 2 * DC * 512], DT_IN, kind="ExternalInput").ap()
    gq_d = nc.dram_tensor("gq", [P, DC * D], DT_IN, kind="ExternalInput").ap()
    xv_d = nc.dram_tensor("xv", [P, SB * D], DT_IN, kind="ExternalInput").ap()
    wv_d = nc.dram_tensor("wv", [P, DC * D], DT_IN, kind="ExternalInput").ap()
    # masks + a trailing 128x128 identity (for mask-injection matmuls)
    mask_d = nc.dram_tensor("mask", [P, (SB + 1) * P], DT_IN,
                            kind="ExternalInput").ap()
    o_d = nc.dram_tensor("o", [P, NQ * D], DT_IN, kind="ExternalOutput").ap()

    for _rep in range(reps):
        _emit_body(nc, xt_d, xtq_d, gq_d, xv_d, wv_d, mask_d, o_d)
    return nc


def _emit_body(nc, xt_d, xtq_d, gq_d, xv_d, wv_d, mask_d, o_d):
    with tile.TileContext(nc) as tc:
        persist = tc.alloc_tile_pool(name="persist", bufs=1)
        tt = [persist.tile([P, QW], DT_IN, tag=f"tt{c}", name=f"tt{c}")
              for c in range(DC)]
        ones = persist.tile([P, P], DT_IN, tag="ones", name="ones")
        nc.gpsimd.memset(ones[:], 1.0)
        # pxt_sb[c][:, 512p:512p+512] = (P~X)^T chunk, normalized, bf16
        pxt_sb = [persist.tile([P, QW], DT_IN, tag=f"px{c}", name=f"px{c}")
                  for c in range(DC)]
        masks = persist.tile([P, (SB + 1) * P], DT_IN, tag="masks", name="masks")

        wv_pool = tc.alloc_tile_pool(name="wv_pool", bufs=1)
        wv = wv_pool.tile([P, DC * D], DT_IN, tag="wv", name="wv")
        xt_pool = tc.alloc_tile_pool(name="xt_pool", bufs=1)
        xt = xt_pool.tile([P, SB * DC * P], DT_IN, tag="xt", name="xt")
        xv_pool = tc.alloc_tile_pool(name="xv_pool", bufs=1)
        xv = xv_pool.tile([P, SB * D], DT_IN, tag="xv", name="xv")
        gq_pool = tc.alloc_tile_pool(name="gq_pool", bufs=1)
        gq = gq_pool.tile([P, DC * D], DT_IN, tag="gq", name="gq")
        xtq_pool = tc.alloc_tile_pool(name="xtq_pool", bufs=1)
        xtq = xtq_pool.tile([P, 2 * DC * 512], DT_IN, tag="xtq", name="xtq")

        def xt_sl(c, j):
            off = (j * DC + c) * P
            return xt[:, off:off + P]

        def xtq_sl(c, g):
            off = (g * DC + c) * 512
            return xtq[:, off:off + 512]

        def gq_sl(co, ci):
            return gq[:, co * D + ci * P:co * D + (ci + 1) * P]

        def xv_sl(j, c):
            return xv[:, j * D + c * P:j * D + (c + 1) * P]

        def wv_sl(c, n0, nw):
            return wv[:, c * D + n0:c * D + n0 + nw]

        def mask_sl(j):
            # masks layout: [identity | m0 | m1 | ... | m15]
            return masks[:, (j + 1) * P:(j + 2) * P]

        ident = masks[:, 0:P]

        # ---------------- input DMAs, three queues, first-use order --------
        # phase T's first matmuls need gq co=0 + xtq (g0, ci=0): put small
        # leading pieces FIRST on two different queues so TT starts early.
        # xt is j-major so pass A's early key blocks land first.
        H3 = 3 * D
        Q = DC * 512
        JW = DC * P                         # one j block of xt
        # sync queue
        nc.sync.dma_start(gq[:, 0:H3], gq_d[:, 0:H3])
        nc.sync.dma_start(gq[:, H3:2 * H3], gq_d[:, H3:2 * H3])
        nc.sync.dma_start(masks[:, 0:9 * P], mask_d[:, 0:9 * P])
        nc.sync.dma_start(xv[:, 0:2 * D], xv_d[:, 0:2 * D])
        nc.sync.dma_start(xv[:, 2 * D:4 * D], xv_d[:, 2 * D:4 * D])
        nc.sync.dma_start(masks[:, 9 * P:], mask_d[:, 9 * P:])
        nc.sync.dma_start(wv[:], wv_d[:])
        # scalar queue
        nc.scalar.dma_start(xtq[:, 0:2 * 512], xtq_d[:, 0:2 * 512])
        nc.scalar.dma_start(xtq[:, 2 * 512:4 * 512], xtq_d[:, 2 * 512:4 * 512])
        nc.scalar.dma_start(xtq[:, 4 * 512:Q], xtq_d[:, 4 * 512:Q])
        nc.scalar.dma_start(xtq[:, Q:Q + 3 * 512], xtq_d[:, Q:Q + 3 * 512])
        nc.scalar.dma_start(xtq[:, Q + 3 * 512:], xtq_d[:, Q + 3 * 512:])
        nc.scalar.dma_start(xt[:, 8 * JW:], xt_d[:, 8 * JW:])
        nc.scalar.dma_start(xv[:, 8 * D:12 * D], xv_d[:, 8 * D:12 * D])
        nc.scalar.dma_start(xv[:, 12 * D:], xv_d[:, 12 * D:])
        # gpsimd (SWDGE) queue
        nc.gpsimd.dma_start(xt[:, 0:4 * JW], xt_d[:, 0:4 * JW])
        nc.gpsimd.dma_start(xt[:, 4 * JW:8 * JW], xt_d[:, 4 * JW:8 * JW])
        nc.gpsimd.dma_start(xv[:, 4 * D:6 * D], xv_d[:, 4 * D:6 * D])
        nc.gpsimd.dma_start(xv[:, 6 * D:8 * D], xv_d[:, 6 * D:8 * D])

        # ---------------- PE warm-up while DMAs land -----------------------
        # ~36 junk matmuls on the memset `ones` tile get HAM to K=8/8
        # (2.4 GHz) before the first real matmul issues.
        with tc.tile_pool(name="psum_w", bufs=1, space="PSUM") as ppw:
            wps = ppw.tile([P, P], F32, tag="warm", name="warm")
            for _ in range(36):
                nc.tensor.matmul(wps[:], ones[:], ones[:], start=True, stop=True)

        # ---------------- Phase T: tt[co][:, g] = sum_ci G^T XTq -----------
        with tc.tile_pool(name="psum_t", bufs=4, space="PSUM") as ppt:
            for g in range(QW // 512):
                for co in range(DC):
                    ps = ppt.tile([P, 512], F32, tag="pt", name="pt")
                    for ci in range(DC):
                        nc.tensor.matmul(
                            ps[:],
                            gq_sl(co, ci),
                            xtq_sl(ci, g),
                            start=(ci == 0), stop=(ci == DC - 1),
                        )
                    if co % 2 == 0:
                        nc.scalar.copy(tt[co][:, g * 512:(g + 1) * 512], ps[:])
                    else:
                        nc.vector.tensor_copy(tt[co][:, g * 512:(g + 1) * 512], ps[:])
        xtq_pool.release()
        gq_pool.release()

        # ---------------- Phase A: scores + exp + PX accumulation ----------
        with (
            tc.tile_pool(name="psum_acc", bufs=1, space="PSUM") as pacc,
            tc.tile_pool(name="psum_st", bufs=1, space="PSUM") as pst,
            tc.tile_pool(name="pt_sb", bufs=3) as pt_pool,
            tc.tile_pool(name="rc_sb", bufs=2) as rc_pool,
        ):
            acc = [pacc.tile([P, 512], F32, tag=f"acc{c}", name=f"acc{c}")
                   for c in range(DC + 1)]

            for p in (0, 1):
                jmax = 8 * p + 7
                base = 512 * p
                pts = {}

                def emit_st(j):
                    s0l = max(0, j // 2 - 4 * p)
                    c0 = s0l * P
                    masked = (p == 0 or j >= 8)
                    st = pst.tile([P, 512], F32, tag="st", name="st")
                    for ci in range(DC):
                        nc.tensor.matmul(
                            st[:, c0:512],
                            xt_sl(ci, j),
                            tt[ci][:, base + c0:base + 512],
                            start=(ci == 0),
                            stop=(ci == DC - 1 and not masked),
                        )
                    if masked:
                        # inject the additive mask on the PE itself:
                        # I.T @ mask == mask, accumulated into the group.
                        nc.tensor.matmul(
                            st[:, c0:c0 + P], ident, mask_sl(j),
                            start=False, stop=True,
                        )
                    ptile = pt_pool.tile([P, 512], DT_IN, tag="ptile", name="ptile")
                    nc.scalar.activation(
                        ptile[:, c0:512], st[:, c0:512],
                        mybir.ActivationFunctionType.Exp, scale=SCALE,
                    )
                    pts[j] = (ptile, c0)

                def emit_px(j):
                    ptile, c0 = pts.pop(j)
                    for c in range(DC + 1):
                        lhs = xv_sl(j, c) if c < DC else ones[:]
                        nc.tensor.matmul(
                            acc[c][:, c0:512], lhs, ptile[:, c0:512],
                            start=(j == 0), stop=(j == jmax),
                        )

                # software pipeline: ST_{j+1} ahead of PX_j so exp_j hides
                # under PX_{j-1} and the single ST bank never stalls PE.
                emit_st(0)
                for j in range(jmax + 1):
                    if j + 1 <= jmax:
                        emit_st(j + 1)
                    emit_px(j)

                # pass end: normalize and stash as bf16 for phase O.
                # c ASCENDING: frees psum banks 0,1,... in the same order
                # phase O's pool allocates them AND produces pxt_sb[0] first,
                # which phase O's (also ascending) groups consume first.
                rc = rc_pool.tile([P, 512], F32, tag="rc", name="rc")
                nc.vector.reciprocal_approx_fast(rc[:], acc[DC][:])
                for c in range(DC):
                    nc.vector.tensor_mul(pxt_sb[c][:, base:base + 512],
                                         acc[c][:], rc[:])

        xv_pool.release()
        xt_pool.release()

        # ---------------- Phase O: O_i = sum_c pxt-chunk @ Wv[c] -----------
        with (
            tc.tile_pool(name="psum_o", bufs=3, space="PSUM") as pso,
            tc.tile_pool(name="o_sb", bufs=3) as o_pool,
        ):
            for i in range(NQ):
                po = pso.tile([P, 1024], F32, tag="po", name="po")
                for (n0, nw) in ((0, 512), (512, D - 512)):
                    for c in range(DC):
                        nc.tensor.matmul(
                            po[:, n0:n0 + nw],
                            pxt_sb[c][:, i * P:(i + 1) * P],
                            wv_sl(c, n0, nw),
                            start=(c == 0), stop=(c == DC - 1),
                        )
                osb = o_pool.tile([P, D], DT_IN, tag="osb", name="osb")
                # split the copy across both psum-capable engines
                nc.scalar.copy(osb[:, 0:384], po[:, 0:384])
                nc.vector.tensor_copy(osb[:, 384:D], po[:, 384:D])
                if i % 2 == 0:
                    nc.sync.dma_start(o_d[:, i * D:(i + 1) * D], osb[:])
                else:
                    nc.scalar.dma_start(o_d[:, i * D:(i + 1) * D], osb[:])

        wv_pool.release()
        persist.release()


# ---------------------------------------------------------------------------
# host side

def _build_masks():
    """masks[r] : [128, 17*128] bf16 = [identity | m0 | ... | m15]. Block j
    is applied (via an I-stationary matmul) to the first 128 q-cols of the
    ST_j window (global q slot j//2, global q block g = 2*(j//2) + r).

    g == j  -> causal tril (allow k <= q)
    g <  j  -> pad slot: kill the whole block
    g >  j  -> fully allowed
    """
    tril = np.tril(np.full((P, P), MASK_VAL, np.float32), -1)  # kill k > q
    out = []
    for r in (0, 1):
        m = np.zeros((SB, P, P), np.float32)
        for j in range(SB):
            g = 2 * (j // 2) + r
            if g == j:
                m[j] = tril
            elif g < j:
                m[j, :, :] = MASK_VAL
        # [j, p, col] -> [p, (1+j)*128+col], identity at cols [0:128]
        import ml_dtypes
        flat = np.concatenate(
            [np.eye(P, dtype=np.float32),
             m.transpose(1, 0, 2).reshape(P, SB * P)], axis=1)
        out.append(np.ascontiguousarray(flat.astype(ml_dtypes.bfloat16)))
    return out


def _perm_rows(a, nch):
    """[nch*128, W] -> [128, nch*W] with chunk c at cols [c*W:(c+1)*W]."""
    w = a.shape[1]
    return np.ascontiguousarray(
        a.reshape(nch, P, w).transpose(1, 0, 2).reshape(P, nch * w))


_STATE = {}


def _get_nc():
    if "nc" not in _STATE:
        nc = build_nc()
        nc.finalize()
        _STATE["nc"] = nc
    return _STATE["nc"]


def _cast_in(a):
    import ml_dtypes
    return np.ascontiguousarray(a.astype(ml_dtypes.bfloat16))


def build_in_maps(x, Wq, Wk, Wv):
    masks = _build_masks()
    G = np.ascontiguousarray(Wq.astype(np.float32) @ Wk.astype(np.float32).T)
    # co-major pack: row block co holds G[ci*128:(ci+1)*128, co*...] tiles.
    gp = np.concatenate(
        [np.concatenate([G[ci * P:(ci + 1) * P, co * P:(co + 1) * P]
                         for ci in range(DC)], axis=1) for co in range(DC)],
        axis=0)
    gqp = _cast_in(_perm_rows(gp, DC))                       # [128, 6*768]
    wvp = _cast_in(_perm_rows(np.asarray(Wv, np.float32), DC))
    in_maps = []
    for b in range(B):
        xb = np.asarray(x[b], np.float32)
        xt = xb.T                                            # [768, 2048]
        # xt j-major: [p, (j*6+c)*128+col] = XT[c*128+p, j*128+col]
        xtp = _cast_in(
            xt.reshape(DC, P, SB, P).transpose(1, 2, 0, 3).reshape(P, -1))
        xvp = _cast_in(_perm_rows(xb, SB))                   # [128, 16*768]
        for r in (0, 1):
            cols = np.concatenate(
                [xt[:, (2 * i + r) * P:(2 * i + r + 1) * P] for i in range(NQ)],
                axis=1,
            )                                                # [768, 1024]
            # xtq: [p, (g*6+c)*512+col] = cols[c*128+p, g*512+col]
            xtqp = _cast_in(
                cols.reshape(DC, P, 2, 512).transpose(1, 2, 0, 3).reshape(P, -1))
            in_maps.append({
                "xt": xtp,
                "xtq": xtqp,
                "gq": gqp, "xv": xvp, "wv": wvp,
                "mask": masks[r],
            })
    return in_maps


def kernel(x, Wq, Wk, Wv):
    x = np.ascontiguousarray(np.asarray(x, np.float32))
    Wq = np.ascontiguousarray(np.asarray(Wq, np.float32))
    Wk = np.ascontiguousarray(np.asarray(Wk, np.float32))
    Wv = np.ascontiguousarray(np.asarray(Wv, np.float32))

    from concourse.bass_utils import run_bass_kernel_spmd

    nc = _get_nc()
    in_maps = build_in_maps(x, Wq, Wk, Wv)

    res = run_bass_kernel_spmd(nc, in_maps, core_ids=list(range(8)), trace=False)

    out = np.empty((B, S, D), np.float32)
    for b in range(B):
        for r in (0, 1):
            o = np.asarray(res.results[2 * b + r]["o"], np.float32)
            # [p, i*768+col] -> rows g=2i+r
            ob = o.reshape(P, NQ, D).transpose(1, 0, 2)      # [8, 128, 768]
            for i in range(NQ):
                g = 2 * i + r
                out[b, g * P:(g + 1) * P, :] = ob[i]
    return out


# ---------------------------------------------------------------------------
# benchmarking support (not used by the grading path)

def _make_executor(nc, n_cores=8):
    """Build a cached jitted SPMD callable (no donation, reusable buffers)."""
    import jax
    from jax.sharding import Mesh, PartitionSpec
    try:
        from jax.experimental.shard_map import shard_map
    except ImportError:
        from jax.shard_map import shard_map
    from concourse import bass2jax
    from concourse import mybir as mb

    bass2jax.install_neuronx_cc_hook()
    partition_name = nc.partition_id_tensor.name if nc.partition_id_tensor else None
    in_names, out_names, out_avals, zero_outs = [], [], [], []
    for alloc in nc.m.functions[0].allocations:
        if not isinstance(alloc, mb.MemoryLocationSet):
            continue
        name = alloc.memorylocations[0].name
        if alloc.kind == "ExternalInput":
            if name != partition_name:
                in_names.append(name)
        elif alloc.kind == "ExternalOutput":
            shape = tuple(alloc.tensor_shape)
            dtype = mb.dt.np(alloc.dtype)
            out_names.append(name)
            out_avals.append(jax.core.ShapedArray(shape, dtype))
            zero_outs.append(np.zeros(shape, dtype))
    n_params = len(in_names)
    all_names = list(in_names) + list(out_names)
    if partition_name is not None:
        all_names.append(partition_name)

    def _body(*args):
        operands = list(args)
        if partition_name is not None:
            operands.append(bass2jax.partition_id_tensor())
        outs = bass2jax._bass_exec_p.bind(
            *operands,
            out_avals=tuple(out_avals),
            in_names=tuple(all_names),
            out_names=tuple(out_names),
            lowering_input_output_aliases=(),
            sim_require_finite=True,
            sim_require_nnan=True,
            nc=nc,
        )
        return tuple(outs)

    devices = jax.devices()[:n_cores]
    mesh = Mesh(np.asarray(devices), ("core",))
    in_specs = (PartitionSpec("core"),) * (n_params + len(out_names))
    out_specs = (PartitionSpec("core"),) * len(out_names)
    sharded = jax.jit(
        shard_map(_body, mesh=mesh, in_specs=in_specs, out_specs=out_specs,
                  check_rep=False),
        keep_unused=True,
    )
    return sharded, in_names, out_names, out_avals, zero_outs


def benchmark(in_maps, iters=20, n_cores=8):
    """Run the compiled kernel `iters` times back-to-back on device-resident
    inputs; returns (per_iter_seconds, results_core0_dict)."""
    import time as _time
    import jax

    nc = _get_nc()
    key = "exec"
    if key not in _STATE:
        _STATE[key] = _make_executor(nc, n_cores)
    sharded, in_names, out_names, out_avals, zero_outs = _STATE[key]

    concat_in = [
        np.concatenate([np.asarray(in_maps[c][n]) for c in range(n_cores)], axis=0)
        for n in in_names
    ]
    concat_zeros = [
        np.zeros((n_cores * z.shape[0], *z.shape[1:]), z.dtype) for z in zero_outs
    ]
    args = [jax.device_put(a) for a in concat_in + concat_zeros]
    jax.block_until_ready(args)

    outs = sharded(*args)          # warm-up / compile
    jax.block_until_ready(outs)
    t0 = _time.time()
    for _ in range(iters):
        outs = sharded(*args)
    jax.block_until_ready(outs)
    per_iter = (_time.time() - t0) / iters
    res0 = {
        n: np.asarray(outs[i]).reshape(n_cores, *out_avals[i].shape)[0]
        for i, n in enumerate(out_names)
    }
    return per_iter, res0


def measure_exec_ns(iters=12, reps_pair=(64, 96)):
    """Estimate true per-core HW execution time of one kernel body.

    Per-call wall time through the axon tunnel is dominated by a transfer
    floor proportional to I/O bytes (~16 ms) that completely hides execution.
    So we build NEFFs with the body repeated r1/r2 times (same I/O footprint)
    and use the slope: (wall(r2) - wall(r1)) / (r2 - r1).
    """
    import time as _time
    import jax

    rng = np.random.default_rng(0)
    x = rng.standard_normal((B, S, D)).astype(np.float32)
    sc = 1.0 / np.sqrt(D)
    Wq = rng.uniform(-sc, sc, (D, D)).astype(np.float32)
    Wk = rng.uniform(-sc, sc, (D, D)).astype(np.float32)
    Wv = rng.uniform(-sc, sc, (D, D)).astype(np.float32)
    in_maps = build_in_maps(x, Wq, Wk, Wv)

    pers = {}
    for reps in reps_pair:
        nc = build_nc(reps=reps)
        nc.finalize()
        sharded, in_names, out_names, out_avals, zero_outs = _make_executor(nc, 8)
        concat_in = [
            np.concatenate([np.asarray(in_maps[c][n]) for c in range(8)], axis=0)
            for n in in_names
        ]
        concat_zeros = [
            np.zeros((8 * z.shape[0], *z.shape[1:]), z.dtype) for z in zero_outs
        ]
        args = [jax.device_put(a) for a in concat_in + concat_zeros]
        jax.block_until_ready(args)
        outs = sharded(*args)
        jax.block_until_ready(outs)
        best = None
        for _ in range(3):
            t0 = _time.time()
            for _ in range(iters):
                outs = sharded(*args)
            jax.block_until_ready(outs)
            per = (_time.time() - t0) / iters
            best = per if best is None else min(best, per)
        pers[reps] = best
    r1, r2 = reps_pair
    return int((pers[r2] - pers[r1]) / (r2 - r1) * 1e9)


# revision 27
# speedup vs baseline: 1.1790x; 1.1790x over previous
"""Causal single-head attention (B=4, S=2048, D=768) on 8 trn2 NeuronCores.

Sharding: batch (4) x query-split (2). Core c = 2*b + r handles batch b and
the 8 interleaved query blocks {2i+r : i=0..7} (128 rows each).

Algebraic restructuring vs the straightforward QKV pipeline:
  scores = (X Wq)(X Wk)^T = X G X^T          with G = Wq Wk^T (host-packed)
  out    = softmax(scores) X Wv              (values = raw X; Wv applied last)
so neither K nor V is ever materialized. Per-core matmul stream drops from
~298k to ~194k PE columns.

All inputs are host-permuted to partition-major [128, N] layouts so each
tensor lands in SBUF with 1-2 large DMA transfers (>=0.6 MB each; small
strided transfers measured ~7x below line rate).

Pipeline per core (matmuls as out = lhsT.T @ rhs, bf16 inputs):
  Phase T : TT[e, q] = G^T @ XTq             (T = Xq G; same cost as Q proj)
  Phase A : two passes over q-halves (slots 0-3, 4-7). Per key block j:
              ST_j[k, q-window] = X_j @ T^T  (xt chunks vs tt chunks)
              +mask on the window's first 128 cols (diag tril / pad kill),
              exp -> PT_j (SBUF bf16)
            then 7 accumulating matmuls into per-d-chunk PSUM banks:
              PXT[d-chunk][:, window] += X_j-chunk-stationary @ PT_j
            chunk 6 uses an all-ones stationary -> replicated row sums.
            Pass end: recip = approx(1/rowsum); pxt_sb[c] = PXT[c] * recip.
  Phase O : O[q-block] = sum_c pxt_sb[c]-chunk-stationary @ Wv[c] -> bf16 out.

Role asymmetry (which key block is diagonal / padded) is carried by the mask
INPUT so the same SPMD program runs on all 8 cores. PSUM: 7 accumulator
banks + 1 ST bank = exactly 8. Variable-width accumulation is legal because
key-block j=0 covers each bank's full 512 cols (start=True clears whole
bank) and later, narrower windows only accumulate (per-element has_written).
"""

import os
import sys

for _p in ("/opt/trn_rl_repo", "/root/.axon_site/_ro/trn_rl_repo"):
    if os.path.isdir(_p) and _p not in sys.path:
        sys.path.append(_p)

import numpy as np

import concourse.bacc as bacc
import concourse.mybir as mybir
import concourse.tile as tile
from concourse._compat import get_trn_type

B, S, D = 4, 2048, 768
P = 128
DC = D // P          # 6 contraction / d chunks
SB = S // P          # 16 seq blocks
NQ = 8               # q-slots per core
QW = NQ * P          # 1024 q rows per core
SCALE = 1.0 / float(np.sqrt(D))
MASK_VAL = -1e30

F32 = mybir.dt.float32
BF16 = mybir.dt.bfloat16
DT_IN = BF16


def build_nc(reps=1):
    nc = bacc.Bacc(
        get_trn_type() or "TRN2",
        target_bir_lowering=False,
        debug=False,
        num_devices=8,
        dynamic_dma_scratch_size=2048,
    )
    # all host-permuted to partition-major [128, N]; see build_in_maps.
    xt_d = nc.dram_tensor("xt", [P, SB * DC * P], DT_IN, kind="ExternalInput").ap()
    xtq_d = nc.dram_tensor("xtq", [P, 2 * DC * 512], DT_IN, kind="ExternalInput").ap()
    gq_d = nc.dram_tensor("gq", [P, DC * D], DT_IN, kind="ExternalInput").ap()
    xv_d = nc.dram_tensor("xv", [P, SB * D], DT_IN, kind="ExternalInput").ap()
    wv_d = nc.dram_tensor("wv", [P, DC * D], DT_IN, kind="ExternalInput").ap()
    # masks + a trailing 128x128 identity (for mask-injection matmuls)
    mask_d = nc.dram_tensor("mask", [P, (SB + 1) * P], DT_IN,
                            kind="ExternalInput").ap()
    o_d = nc.dram_tensor("o", [P, NQ * D], DT_IN, kind="ExternalOutput").ap()

    for _rep in range(reps):
        _emit_body(nc, xt_d, xtq_d, gq_d, xv_d, wv_d, mask_d, o_d)
    return nc


def _emit_body(nc, xt_d, xtq_d, gq_d, xv_d, wv_d, mask_d, o_d):
    with tile.TileContext(nc) as tc:
        persist = tc.alloc_tile_pool(name="persist", bufs=1)
        tt = [persist.tile([P, QW], DT_IN, tag=f"tt{c}", name=f"tt{c}")
              for c in range(DC)]
        ones = persist.tile([P, P], DT_IN, tag="ones", name="ones")
        nc.gpsimd.memset(ones[:], 1.0)
        # pxt_sb[c][:, 512p:512p+512] = (P~X)^T chunk, normalized, bf16
        pxt_sb = [persist.tile([P, QW], DT_IN, tag=f"px{c}", name=f"px{c}")
                  for c in range(DC)]
        masks = persist.tile([P, (SB + 1) * P], DT_IN, tag="masks", name="masks")

        wv_pool = tc.alloc_tile_pool(name="wv_pool", bufs=1)
        wv = wv_pool.tile([P, DC * D], DT_IN, tag="wv", name="wv")
        xt_pool = tc.alloc_tile_pool(name="xt_pool", bufs=1)
        xt = xt_pool.tile([P, SB * DC * P], DT_IN, tag="xt", name="xt")
        xv_pool = tc.alloc_tile_pool(name="xv_pool", bufs=1)
        xv = xv_pool.tile([P, SB * D], DT_IN, tag="xv", name="xv")
        gq_pool = tc.alloc_tile_pool(name="gq_pool", bufs=1)
        gq = gq_pool.tile([P, DC * D], DT_IN, tag="gq", name="gq")
        xtq_pool = tc.alloc_tile_pool(name="xtq_pool", bufs=1)
        xtq = xtq_pool.tile([P, 2 * DC * 512], DT_IN, tag="xtq", name="xtq")

        def xt_sl(c, j):
            off = (j * DC + c) * P
            return xt[:, off:off + P]

        def xtq_sl(c, g):
            off = (g * DC + c) * 512
            return xtq[:, off:off + 512]

        def gq_sl(co, ci):
            return gq[:, co * D + ci * P:co * D + (ci + 1) * P]

        def xv_sl(j, c):
            return xv[:, j * D + c * P:j * D + (c + 1) * P]

        def wv_sl(c, n0, nw):
            return wv[:, c * D + n0:c * D + n0 + nw]

        def mask_sl(j):
            # masks layout: [identity | m0 | m1 | ... | m15]
            return masks[:, (j + 1) * P:(j + 2) * P]

        ident = masks[:, 0:P]

        # ---------------- input DMAs, three queues, first-use order --------
        # phase T's first matmuls need gq co=0 + xtq (g0, ci=0): put small
        # leading pieces FIRST on two different queues so TT starts early.
        # xt is j-major so pass A's early key blocks land first.
        H3 = 3 * D
        Q = DC * 512
        JW = DC * P                         # one j block of xt
        # sync queue
        nc.sync.dma_start(gq[:, 0:H3], gq_d[:, 0:H3])
        nc.sync.dma_start(gq[:, H3:2 * H3], gq_d[:, H3:2 * H3])
        nc.sync.dma_start(masks[:, 0:9 * P], mask_d[:, 0:9 * P])
        nc.sync.dma_start(xv[:, 0:2 * D], xv_d[:, 0:2 * D])
        nc.sync.dma_start(xv[:, 2 * D:4 * D], xv_d[:, 2 * D:4 * D])
        nc.sync.dma_start(masks[:, 9 * P:], mask_d[:, 9 * P:])
        nc.sync.dma_start(wv[:], wv_d[:])
        # scalar queue
        nc.scalar.dma_start(xtq[:, 0:2 * 512], xtq_d[:, 0:2 * 512])
        nc.scalar.dma_start(xtq[:, 2 * 512:4 * 512], xtq_d[:, 2 * 512:4 * 512])
        nc.scalar.dma_start(xtq[:, 4 * 512:Q], xtq_d[:, 4 * 512:Q])
        nc.scalar.dma_start(xtq[:, Q:Q + 3 * 512], xtq_d[:, Q:Q + 3 * 512])
        nc.scalar.dma_start(xtq[:, Q + 3 * 512:], xtq_d[:, Q + 3 * 512:])
        nc.scalar.dma_start(xt[:, 8 * JW:], xt_d[:, 8 * JW:])
        nc.scalar.dma_start(xv[:, 8 * D:12 * D], xv_d[:, 8 * D:12 * D])
        nc.scalar.dma_start(xv[:, 12 * D:], xv_d[:, 12 * D:])
        # gpsimd (SWDGE) queue
        nc.gpsimd.dma_start(xt[:, 0:4 * JW], xt_d[:, 0:4 * JW])
        nc.gpsimd.dma_start(xt[:, 4 * JW:8 * JW], xt_d[:, 4 * JW:8 * JW])
        nc.gpsimd.dma_start(xv[:, 4 * D:6 * D], xv_d[:, 4 * D:6 * D])
        nc.gpsimd.dma_start(xv[:, 6 * D:8 * D], xv_d[:, 6 * D:8 * D])

        # ---------------- PE warm-up while DMAs land -----------------------
        # ~36 junk matmuls on the memset `ones` tile get HAM to K=8/8
        # (2.4 GHz) before the first real matmul issues.
        with tc.tile_pool(name="psum_w", bufs=1, space="PSUM") as ppw:
            wps = ppw.tile([P, P], F32, tag="warm", name="warm")
            for _ in range(36):
                nc.tensor.matmul(wps[:], ones[:], ones[:], start=True, stop=True)

        # ---------------- Phase T: tt[co][:, g] = sum_ci G^T XTq -----------
        with tc.tile_pool(name="psum_t", bufs=4, space="PSUM") as ppt:
            for g in range(QW // 512):
                for co in range(DC):
                    ps = ppt.tile([P, 512], F32, tag="pt", name="pt")
                    for ci in range(DC):
                        nc.tensor.matmul(
                            ps[:],
                            gq_sl(co, ci),
                            xtq_sl(ci, g),
                            start=(ci == 0), stop=(ci == DC - 1),
                        )
                    if co % 2 == 0:
                        nc.scalar.copy(tt[co][:, g * 512:(g + 1) * 512], ps[:])
                    else:
                        nc.vector.tensor_copy(tt[co][:, g * 512:(g + 1) * 512], ps[:])
        xtq_pool.release()
        gq_pool.release()

        # ---------------- Phase A: scores + exp + PX accumulation ----------
        with (
            tc.tile_pool(name="psum_acc", bufs=1, space="PSUM") as pacc,
            tc.tile_pool(name="psum_st", bufs=1, space="PSUM") as pst,
            tc.tile_pool(name="pt_sb", bufs=3) as pt_pool,
            tc.tile_pool(name="rc_sb", bufs=2) as rc_pool,
        ):
            acc = [pacc.tile([P, 512], F32, tag=f"acc{c}", name=f"acc{c}")
                   for c in range(DC + 1)]

            for p in (0, 1):
                jmax = 8 * p + 7
                base = 512 * p
                pts = {}

                def emit_st(j):
                    s0l = max(0, j // 2 - 4 * p)
                    c0 = s0l * P
                    masked = (p == 0 or j >= 8)
                    st = pst.tile([P, 512], F32, tag="st", name="st")
                    for ci in range(DC):
                        nc.tensor.matmul(
                            st[:, c0:512],
                            xt_sl(ci, j),
                            tt[ci][:, base + c0:base + 512],
                            start=(ci == 0),
                            stop=(ci == DC - 1 and not masked),
                        )
                    if masked:
                        # inject the additive mask on the PE itself:
                        # I.T @ mask == mask, accumulated into the group.
                        nc.tensor.matmul(
                            st[:, c0:c0 + P], ident, mask_sl(j),
                            start=False, stop=True,
                        )
                    ptile = pt_pool.tile([P, 512], DT_IN, tag="ptile", name="ptile")
                    nc.scalar.activation(
                        ptile[:, c0:512], st[:, c0:512],
                        mybir.ActivationFunctionType.Exp, scale=SCALE,
                    )
                    pts[j] = (ptile, c0)

                def emit_px(j):
                    ptile, c0 = pts.pop(j)
                    # c DESCENDING (E-chunk first): at the pass A -> pass B
                    # boundary the banks free in this order (recip on acc[6]
                    # first, then the descending muls), so j=0's start=True
                    # writes chase them with minimal stall.
                    for c in reversed(range(DC + 1)):
                        lhs = xv_sl(j, c) if c < DC else ones[:]
                        nc.tensor.matmul(
                            acc[c][:, c0:512], lhs, ptile[:, c0:512],
                            start=(j == 0), stop=(j == jmax),
                        )

                # software pipeline: ST_{j+1} ahead of PX_j so exp_j hides
                # under PX_{j-1} and the single ST bank never stalls PE.
                emit_st(0)
                for j in range(jmax + 1):
                    if j + 1 <= jmax:
                        emit_st(j + 1)
                    emit_px(j)

                # pass end: normalize and stash as bf16 for phase O.
                # c DESCENDING so banks 5,4,... free early for phase O's
                # right-side psum pool (buf1 = banks 4,5). Phase O's first
                # four q-blocks consume PASS A data, which is long done, so
                # the pass-B muls only gate blocks 4-7.
                rc = rc_pool.tile([P, 512], F32, tag="rc", name="rc")
                nc.vector.reciprocal_approx_fast(rc[:], acc[DC][:])
                for c in reversed(range(DC)):
                    nc.vector.tensor_mul(pxt_sb[c][:, base:base + 512],
                                         acc[c][:], rc[:])

        xv_pool.release()
        xt_pool.release()

        # ---------------- Phase O: O_i = sum_c pxt-chunk @ Wv[c] -----------
        with (
            tc.tile_pool(name="psum_o", bufs=3, space="PSUM", side="right") as pso,
            tc.tile_pool(name="o_sb", bufs=3) as o_pool,
        ):
            for i in range(NQ):
                po = pso.tile([P, 1024], F32, tag="po", name="po")
                for (n0, nw) in ((0, 512), (512, D - 512)):
                    for c in range(DC):
                        nc.tensor.matmul(
                            po[:, n0:n0 + nw],
                            pxt_sb[c][:, i * P:(i + 1) * P],
                            wv_sl(c, n0, nw),
                            start=(c == 0), stop=(c == DC - 1),
                        )
                osb = o_pool.tile([P, D], DT_IN, tag="osb", name="osb")
                # split the copy across both psum-capable engines
                nc.scalar.copy(osb[:, 0:384], po[:, 0:384])
                nc.vector.tensor_copy(osb[:, 384:D], po[:, 384:D])
                if i % 2 == 0:
                    nc.sync.dma_start(o_d[:, i * D:(i + 1) * D], osb[:])
                else:
                    nc.scalar.dma_start(o_d[:, i * D:(i + 1) * D], osb[:])

        wv_pool.release()
        persist.release()


# ---------------------------------------------------------------------------
# host side

def _build_masks():
    """masks[r] : [128, 17*128] bf16 = [identity | m0 | ... | m15]. Block j
    is applied (via an I-stationary matmul) to the first 128 q-cols of the
    ST_j window (global q slot j//2, global q block g = 2*(j//2) + r).

    g == j  -> causal tril (allow k <= q)
    g <  j  -> pad slot: kill the whole block
    g >  j  -> fully allowed
    """
    tril = np.tril(np.full((P, P), MASK_VAL, np.float32), -1)  # kill k > q
    out = []
    for r in (0, 1):
        m = np.zeros((SB, P, P), np.float32)
        for j in range(SB):
            g = 2 * (j // 2) + r
            if g == j:
                m[j] = tril
            elif g < j:
                m[j, :, :] = MASK_VAL
        # [j, p, col] -> [p, (1+j)*128+col], identity at cols [0:128]
        import ml_dtypes
        flat = np.concatenate(
            [np.eye(P, dtype=np.float32),
             m.transpose(1, 0, 2).reshape(P, SB * P)], axis=1)
        out.append(np.ascontiguousarray(flat.astype(ml_dtypes.bfloat16)))
    return out


def _perm_rows(a, nch):
    """[nch*128, W] -> [128, nch*W] with chunk c at cols [c*W:(c+1)*W]."""
    w = a.shape[1]
    return np.ascontiguousarray(
        a.reshape(nch, P, w).transpose(1, 0, 2).reshape(P, nch * w))


_STATE = {}


def _get_nc():
    if "nc" not in _STATE:
        nc = build_nc()
        nc.finalize()
        _STATE["nc"] = nc
    return _STATE["nc"]


def _cast_in(a):
    import ml_dtypes
    return np.ascontiguousarray(a.astype(ml_dtypes.bfloat16))


def build_in_maps(x, Wq, Wk, Wv):
    masks = _build_masks()
    G = np.ascontiguousarray(Wq.astype(np.float32) @ Wk.astype(np.float32).T)
    # co-major pack: row block co holds G[ci*128:(ci+1)*128, co*...] tiles.
    gp = np.concatenate(
        [np.concatenate([G[ci * P:(ci + 1) * P, co * P:(co + 1) * P]
                         for ci in range(DC)], axis=1) for co in range(DC)],
        axis=0)
    gqp = _cast_in(_perm_rows(gp, DC))                       # [128, 6*768]
    wvp = _cast_in(_perm_rows(np.asarray(Wv, np.float32), DC))
    in_maps = []
    for b in range(B):
        xb = np.asarray(x[b], np.float32)
        xt = xb.T                                            # [768, 2048]
        # xt j-major: [p, (j*6+c)*128+col] = XT[c*128+p, j*128+col]
        xtp = _cast_in(
            xt.reshape(DC, P, SB, P).transpose(1, 2, 0, 3).reshape(P, -1))
        xvp = _cast_in(_perm_rows(xb, SB))                   # [128, 16*768]
        for r in (0, 1):
            cols = np.concatenate(
                [xt[:, (2 * i + r) * P:(2 * i + r + 1) * P] for i in range(NQ)],
                axis=1,
            )                                                # [768, 1024]
            # xtq: [p, (g*6+c)*512+col] = cols[c*128+p, g*512+col]
            xtqp = _cast_in(
                cols.reshape(DC, P, 2, 512).transpose(1, 2, 0, 3).reshape(P, -1))
            in_maps.append({
                "xt": xtp,
                "xtq": xtqp,
                "gq": gqp, "xv": xvp, "wv": wvp,
                "mask": masks[r],
            })
    return in_maps


def kernel(x, Wq, Wk, Wv):
    x = np.ascontiguousarray(np.asarray(x, np.float32))
    Wq = np.ascontiguousarray(np.asarray(Wq, np.float32))
    Wk = np.ascontiguousarray(np.asarray(Wk, np.float32))
    Wv = np.ascontiguousarray(np.asarray(Wv, np.float32))

    from concourse.bass_utils import run_bass_kernel_spmd

    nc = _get_nc()
    in_maps = build_in_maps(x, Wq, Wk, Wv)

    res = run_bass_kernel_spmd(nc, in_maps, core_ids=list(range(8)), trace=False)

    out = np.empty((B, S, D), np.float32)
    for b in range(B):
        for r in (0, 1):
            o = np.asarray(res.results[2 * b + r]["o"], np.float32)
            # [p, i*768+col] -> rows g=2i+r
            ob = o.reshape(P, NQ, D).transpose(1, 0, 2)      # [8, 128, 768]
            for i in range(NQ):
                g = 2 * i + r
                out[b, g * P:(g + 1) * P, :] = ob[i]
    return out


# ---------------------------------------------------------------------------
# benchmarking support (not used by the grading path)

def _make_executor(nc, n_cores=8):
    """Build a cached jitted SPMD callable (no donation, reusable buffers)."""
    import jax
    from jax.sharding import Mesh, PartitionSpec
    try:
        from jax.experimental.shard_map import shard_map
    except ImportError:
        from jax.shard_map import shard_map
    from concourse import bass2jax
    from concourse import mybir as mb

    bass2jax.install_neuronx_cc_hook()
    partition_name = nc.partition_id_tensor.name if nc.partition_id_tensor else None
    in_names, out_names, out_avals, zero_outs = [], [], [], []
    for alloc in nc.m.functions[0].allocations:
        if not isinstance(alloc, mb.MemoryLocationSet):
            continue
        name = alloc.memorylocations[0].name
        if alloc.kind == "ExternalInput":
            if name != partition_name:
                in_names.append(name)
        elif alloc.kind == "ExternalOutput":
            shape = tuple(alloc.tensor_shape)
            dtype = mb.dt.np(alloc.dtype)
            out_names.append(name)
            out_avals.append(jax.core.ShapedArray(shape, dtype))
            zero_outs.append(np.zeros(shape, dtype))
    n_params = len(in_names)
    all_names = list(in_names) + list(out_names)
    if partition_name is not None:
        all_names.append(partition_name)

    def _body(*args):
        operands = list(args)
        if partition_name is not None:
            operands.append(bass2jax.partition_id_tensor())
        outs = bass2jax._bass_exec_p.bind(
            *operands,
            out_avals=tuple(out_avals),
            in_names=tuple(all_names),
            out_names=tuple(out_names),
            lowering_input_output_aliases=(),
            sim_require_finite=True,
            sim_require_nnan=True,
            nc=nc,
        )
        return tuple(outs)

    devices = jax.devices()[:n_cores]
    mesh = Mesh(np.asarray(devices), ("core",))
    in_specs = (PartitionSpec("core"),) * (n_params + len(out_names))
    out_specs = (PartitionSpec("core"),) * len(out_names)
    sharded = jax.jit(
        shard_map(_body, mesh=mesh, in_specs=in_specs, out_specs=out_specs,
                  check_rep=False),
        keep_unused=True,
    )
    return sharded, in_names, out_names, out_avals, zero_outs


def benchmark(in_maps, iters=20, n_cores=8):
    """Run the compiled kernel `iters` times back-to-back on device-resident
    inputs; returns (per_iter_seconds, results_core0_dict)."""
    import time as _time
    import jax

    nc = _get_nc()
    key = "exec"
    if key not in _STATE:
        _STATE[key] = _make_executor(nc, n_cores)
    sharded, in_names, out_names, out_avals, zero_outs = _STATE[key]

    concat_in = [
        np.concatenate([np.asarray(in_maps[c][n]) for c in range(n_cores)], axis=0)
        for n in in_names
    ]
    concat_zeros = [
        np.zeros((n_cores * z.shape[0], *z.shape[1:]), z.dtype) for z in zero_outs
    ]
    args = [jax.device_put(a) for a in concat_in + concat_zeros]
    jax.block_until_ready(args)

    outs = sharded(*args)          # warm-up / compile
    jax.block_until_ready(outs)
    t0 = _time.time()
    for _ in range(iters):
        outs = sharded(*args)
    jax.block_until_ready(outs)
    per_iter = (_time.time() - t0) / iters
    res0 = {
        n: np.asarray(outs[i]).reshape(n_cores, *out_avals[i].shape)[0]
        for i, n in enumerate(out_names)
    }
    return per_iter, res0


def measure_exec_ns(iters=12, reps_pair=(64, 96)):
    """Estimate true per-core HW execution time of one kernel body.

    Per-call wall time through the axon tunnel is dominated by a transfer
    floor proportional to I/O bytes (~16 ms) that completely hides execution.
    So we build NEFFs with the body repeated r1/r2 times (same I/O footprint)
    and use the slope: (wall(r2) - wall(r1)) / (r2 - r1).
    """
    import time as _time
    import jax

    rng = np.random.default_rng(0)
    x = rng.standard_normal((B, S, D)).astype(np.float32)
    sc = 1.0 / np.sqrt(D)
    Wq = rng.uniform(-sc, sc, (D, D)).astype(np.float32)
    Wk = rng.uniform(-sc, sc, (D, D)).astype(np.float32)
    Wv = rng.uniform(-sc, sc, (D, D)).astype(np.float32)
    in_maps = build_in_maps(x, Wq, Wk, Wv)

    pers = {}
    for reps in reps_pair:
        nc = build_nc(reps=reps)
        nc.finalize()
        sharded, in_names, out_names, out_avals, zero_outs = _make_executor(nc, 8)
        concat_in = [
            np.concatenate([np.asarray(in_maps[c][n]) for c in range(8)], axis=0)
            for n in in_names
        ]
        concat_zeros = [
            np.zeros((8 * z.shape[0], *z.shape[1:]), z.dtype) for z in zero_outs
        ]
        args = [jax.device_put(a) for a in concat_in + concat_zeros]
        jax.block_until_ready(args)
        outs = sharded(*args)
        jax.block_until_ready(outs)
        best = None
        for _ in range(3):
            t0 = _time.time()
            for _ in range(iters):
                outs = sharded(*args)
            jax.block_until_ready(outs)
            per = (_time.time() - t0) / iters
            best = per if best is None else min(best, per)
        pers[reps] = best
    r1, r2 = reps_pair
    return int((pers[r2] - pers[r1]) / (r2 - r1) * 1e9)


# revision 30
# speedup vs baseline: 1.1824x; 1.0029x over previous
"""Causal single-head attention (B=4, S=2048, D=768) on 8 trn2 NeuronCores.

Sharding: batch (4) x query-split (2). Core c = 2*b + r handles batch b and
the 8 interleaved query blocks {2i+r : i=0..7} (128 rows each).

Algebraic restructuring vs the straightforward QKV pipeline:
  scores = (X Wq)(X Wk)^T = X G X^T          with G = Wq Wk^T (host-packed)
  out    = softmax(scores) X Wv              (values = raw X; Wv applied last)
so neither K nor V is ever materialized. Per-core matmul stream drops from
~298k to ~194k PE columns.

All inputs are host-permuted to partition-major [128, N] layouts so each
tensor lands in SBUF with 1-2 large DMA transfers (>=0.6 MB each; small
strided transfers measured ~7x below line rate).

Pipeline per core (matmuls as out = lhsT.T @ rhs, bf16 inputs):
  Phase T : TT[e, q] = G^T @ XTq             (T = Xq G; same cost as Q proj)
  Phase A : two passes over q-halves (slots 0-3, 4-7). Per key block j:
              ST_j[k, q-window] = X_j @ T^T  (xt chunks vs tt chunks)
              +mask on the window's first 128 cols (diag tril / pad kill),
              exp -> PT_j (SBUF bf16)
            then 7 accumulating matmuls into per-d-chunk PSUM banks:
              PXT[d-chunk][:, window] += X_j-chunk-stationary @ PT_j
            chunk 6 uses an all-ones stationary -> replicated row sums.
            Pass end: recip = approx(1/rowsum); pxt_sb[c] = PXT[c] * recip.
  Phase O : O[q-block] = sum_c pxt_sb[c]-chunk-stationary @ Wv[c] -> bf16 out.

Role asymmetry (which key block is diagonal / padded) is carried by the mask
INPUT so the same SPMD program runs on all 8 cores. PSUM: 7 accumulator
banks + 1 ST bank = exactly 8. Variable-width accumulation is legal because
key-block j=0 covers each bank's full 512 cols (start=True clears whole
bank) and later, narrower windows only accumulate (per-element has_written).
"""

import os
import sys

for _p in ("/opt/trn_rl_repo", "/root/.axon_site/_ro/trn_rl_repo"):
    if os.path.isdir(_p) and _p not in sys.path:
        sys.path.append(_p)

import numpy as np

import concourse.bacc as bacc
import concourse.mybir as mybir
import concourse.tile as tile
from concourse._compat import get_trn_type

B, S, D = 4, 2048, 768
P = 128
DC = D // P          # 6 contraction / d chunks
SB = S // P          # 16 seq blocks
NQ = 8               # q-slots per core
QW = NQ * P          # 1024 q rows per core
SCALE = 1.0 / float(np.sqrt(D))
MASK_VAL = -1e30

F32 = mybir.dt.float32
BF16 = mybir.dt.bfloat16
DT_IN = BF16


def build_nc(reps=1):
    nc = bacc.Bacc(
        get_trn_type() or "TRN2",
        target_bir_lowering=False,
        debug=False,
        num_devices=8,
        dynamic_dma_scratch_size=2048,
    )
    # all host-permuted to partition-major [128, N]; see build_in_maps.
    xt_d = nc.dram_tensor("xt", [P, SB * DC * P], DT_IN, kind="ExternalInput").ap()
    xtq_d = nc.dram_tensor("xtq", [P, 2 * DC * 512], DT_IN, kind="ExternalInput").ap()
    gq_d = nc.dram_tensor("gq", [P, DC * D], DT_IN, kind="ExternalInput").ap()
    xv_d = nc.dram_tensor("xv", [P, SB * D], DT_IN, kind="ExternalInput").ap()
    wv_d = nc.dram_tensor("wv", [P, DC * D], DT_IN, kind="ExternalInput").ap()
    # masks + a trailing 128x128 identity (for mask-injection matmuls)
    mask_d = nc.dram_tensor("mask", [P, (SB + 1) * P], DT_IN,
                            kind="ExternalInput").ap()
    o_d = nc.dram_tensor("o", [P, NQ * D], DT_IN, kind="ExternalOutput").ap()

    for _rep in range(reps):
        _emit_body(nc, xt_d, xtq_d, gq_d, xv_d, wv_d, mask_d, o_d)
    return nc


def _emit_body(nc, xt_d, xtq_d, gq_d, xv_d, wv_d, mask_d, o_d):
    with tile.TileContext(nc) as tc:
        persist = tc.alloc_tile_pool(name="persist", bufs=1)
        tt = [persist.tile([P, QW], DT_IN, tag=f"tt{c}", name=f"tt{c}")
              for c in range(DC)]
        ones = persist.tile([P, P], DT_IN, tag="ones", name="ones")
        nc.gpsimd.memset(ones[:], 1.0)
        # pxt_sb[pass][c] = (P~X)^T chunk for that q-half, normalized, bf16.
        # Separate tiles per pass so phase O's reads of pass-A data don't
        # false-depend on pass-B writes (tile-granular dependency tracking).
        pxt_sb = [[persist.tile([P, 512], DT_IN, tag=f"px{p}_{c}",
                                name=f"px{p}_{c}")
                   for c in range(DC)] for p in (0, 1)]
        masks = persist.tile([P, (SB + 1) * P], DT_IN, tag="masks", name="masks")

        wv_pool = tc.alloc_tile_pool(name="wv_pool", bufs=1)
        wv = wv_pool.tile([P, DC * D], DT_IN, tag="wv", name="wv")
        xt_pool = tc.alloc_tile_pool(name="xt_pool", bufs=1)
        xt = xt_pool.tile([P, SB * DC * P], DT_IN, tag="xt", name="xt")
        xv_pool = tc.alloc_tile_pool(name="xv_pool", bufs=1)
        xv = xv_pool.tile([P, SB * D], DT_IN, tag="xv", name="xv")
        gq_pool = tc.alloc_tile_pool(name="gq_pool", bufs=1)
        gq = gq_pool.tile([P, DC * D], DT_IN, tag="gq", name="gq")
        xtq_pool = tc.alloc_tile_pool(name="xtq_pool", bufs=1)
        xtq = xtq_pool.tile([P, 2 * DC * 512], DT_IN, tag="xtq", name="xtq")

        def xt_sl(c, j):
            off = (j * DC + c) * P
            return xt[:, off:off + P]

        def xtq_sl(c, g):
            off = (g * DC + c) * 512
            return xtq[:, off:off + 512]

        def gq_sl(co, ci):
            return gq[:, co * D + ci * P:co * D + (ci + 1) * P]

        def xv_sl(j, c):
            return xv[:, j * D + c * P:j * D + (c + 1) * P]

        def wv_sl(c, n0, nw):
            return wv[:, c * D + n0:c * D + n0 + nw]

        def mask_sl(j):
            # masks layout: [identity | m0 | m1 | ... | m15]
            return masks[:, (j + 1) * P:(j + 2) * P]

        ident = masks[:, 0:P]

        # ---------------- input DMAs, three queues, first-use order --------
        # phase T's first matmuls need gq co=0 + xtq (g0, ci=0): put small
        # leading pieces FIRST on two different queues so TT starts early.
        # xt is j-major so pass A's early key blocks land first.
        H3 = 3 * D
        Q = DC * 512
        JW = DC * P                         # one j block of xt
        # sync queue
        nc.sync.dma_start(gq[:, 0:H3], gq_d[:, 0:H3])
        nc.sync.dma_start(gq[:, H3:2 * H3], gq_d[:, H3:2 * H3])
        nc.sync.dma_start(masks[:, 0:9 * P], mask_d[:, 0:9 * P])
        nc.sync.dma_start(xv[:, 0:2 * D], xv_d[:, 0:2 * D])
        nc.sync.dma_start(xv[:, 2 * D:4 * D], xv_d[:, 2 * D:4 * D])
        nc.sync.dma_start(masks[:, 9 * P:], mask_d[:, 9 * P:])
        nc.sync.dma_start(wv[:], wv_d[:])
        # scalar queue
        nc.scalar.dma_start(xtq[:, 0:2 * 512], xtq_d[:, 0:2 * 512])
        nc.scalar.dma_start(xtq[:, 2 * 512:4 * 512], xtq_d[:, 2 * 512:4 * 512])
        nc.scalar.dma_start(xtq[:, 4 * 512:Q], xtq_d[:, 4 * 512:Q])
        nc.scalar.dma_start(xtq[:, Q:Q + 3 * 512], xtq_d[:, Q:Q + 3 * 512])
        nc.scalar.dma_start(xtq[:, Q + 3 * 512:], xtq_d[:, Q + 3 * 512:])
        nc.scalar.dma_start(xt[:, 8 * JW:], xt_d[:, 8 * JW:])
        nc.scalar.dma_start(xv[:, 8 * D:12 * D], xv_d[:, 8 * D:12 * D])
        nc.scalar.dma_start(xv[:, 12 * D:], xv_d[:, 12 * D:])
        # gpsimd (SWDGE) queue
        nc.gpsimd.dma_start(xt[:, 0:4 * JW], xt_d[:, 0:4 * JW])
        nc.gpsimd.dma_start(xt[:, 4 * JW:8 * JW], xt_d[:, 4 * JW:8 * JW])
        nc.gpsimd.dma_start(xv[:, 4 * D:6 * D], xv_d[:, 4 * D:6 * D])
        nc.gpsimd.dma_start(xv[:, 6 * D:8 * D], xv_d[:, 6 * D:8 * D])

        # ---------------- PE warm-up while DMAs land -----------------------
        # ~36 junk matmuls on the memset `ones` tile get HAM to K=8/8
        # (2.4 GHz) before the first real matmul issues.
        with tc.tile_pool(name="psum_w", bufs=1, space="PSUM") as ppw:
            wps = ppw.tile([P, P], F32, tag="warm", name="warm")
            for _ in range(36):
                nc.tensor.matmul(wps[:], ones[:], ones[:], start=True, stop=True)

        # ---------------- Phase T: tt[co][:, g] = sum_ci G^T XTq -----------
        with tc.tile_pool(name="psum_t", bufs=4, space="PSUM") as ppt:
            for g in range(QW // 512):
                for co in range(DC):
                    ps = ppt.tile([P, 512], F32, tag="pt", name="pt")
                    for ci in range(DC):
                        nc.tensor.matmul(
                            ps[:],
                            gq_sl(co, ci),
                            xtq_sl(ci, g),
                            start=(ci == 0), stop=(ci == DC - 1),
                        )
                    if co % 2 == 0:
                        nc.scalar.copy(tt[co][:, g * 512:(g + 1) * 512], ps[:])
                    else:
                        nc.vector.tensor_copy(tt[co][:, g * 512:(g + 1) * 512], ps[:])
        xtq_pool.release()
        gq_pool.release()

        # ---------------- Phase A: scores + exp + PX accumulation ----------
        with (
            tc.tile_pool(name="psum_acc", bufs=1, space="PSUM") as pacc,
            tc.tile_pool(name="psum_st", bufs=1, space="PSUM") as pst,
            tc.tile_pool(name="pt_sb", bufs=3) as pt_pool,
            tc.tile_pool(name="rc_sb", bufs=2) as rc_pool,
        ):
            acc = [pacc.tile([P, 512], F32, tag=f"acc{c}", name=f"acc{c}")
                   for c in range(DC + 1)]

            for p in (0, 1):
                jmax = 8 * p + 7
                base = 512 * p
                pts = {}

                def emit_st(j):
                    s0l = max(0, j // 2 - 4 * p)
                    c0 = s0l * P
                    masked = (p == 0 or j >= 8)
                    st = pst.tile([P, 512], F32, tag="st", name="st")
                    for ci in range(DC):
                        nc.tensor.matmul(
                            st[:, c0:512],
                            xt_sl(ci, j),
                            tt[ci][:, base + c0:base + 512],
                            start=(ci == 0),
                            stop=(ci == DC - 1 and not masked),
                        )
                    if masked:
                        # inject the additive mask on the PE itself:
                        # I.T @ mask == mask, accumulated into the group.
                        nc.tensor.matmul(
                            st[:, c0:c0 + P], ident, mask_sl(j),
                            start=False, stop=True,
                        )
                    ptile = pt_pool.tile([P, 512], DT_IN, tag="ptile", name="ptile")
                    nc.scalar.activation(
                        ptile[:, c0:512], st[:, c0:512],
                        mybir.ActivationFunctionType.Exp, scale=SCALE,
                    )
                    pts[j] = (ptile, c0)

                def emit_px(j):
                    ptile, c0 = pts.pop(j)
                    # c DESCENDING (E-chunk first): at the pass A -> pass B
                    # boundary the banks free in this order (recip on acc[6]
                    # first, then the descending muls), so j=0's start=True
                    # writes chase them with minimal stall.
                    for c in reversed(range(DC + 1)):
                        lhs = xv_sl(j, c) if c < DC else ones[:]
                        nc.tensor.matmul(
                            acc[c][:, c0:512], lhs, ptile[:, c0:512],
                            start=(j == 0), stop=(j == jmax),
                        )

                # software pipeline: ST_{j+1} ahead of PX_j so exp_j hides
                # under PX_{j-1} and the single ST bank never stalls PE.
                emit_st(0)
                for j in range(jmax + 1):
                    if j + 1 <= jmax:
                        emit_st(j + 1)
                    emit_px(j)

                # pass end: normalize and stash as bf16 for phase O.
                # c DESCENDING so banks 5,4,... free early: pass B's j=0
                # writes (and phase O's right-side psum pool) chase them.
                rc = rc_pool.tile([P, 512], F32, tag="rc", name="rc")
                nc.vector.reciprocal_approx_fast(rc[:], acc[DC][:])
                for c in reversed(range(DC)):
                    nc.vector.tensor_mul(pxt_sb[p][c][:], acc[c][:], rc[:])

        xv_pool.release()
        xt_pool.release()

        # ---------------- Phase O: O_i = sum_c pxt-chunk @ Wv[c] -----------
        with (
            tc.tile_pool(name="psum_o", bufs=3, space="PSUM", side="right") as pso,
            tc.tile_pool(name="o_sb", bufs=3) as o_pool,
        ):
            for i in range(NQ):
                po = pso.tile([P, 1024], F32, tag="po", name="po")
                for (n0, nw) in ((0, 512), (512, D - 512)):
                    for c in range(DC):
                        nc.tensor.matmul(
                            po[:, n0:n0 + nw],
                            pxt_sb[i // 4][c][:, (i % 4) * P:(i % 4 + 1) * P],
                            wv_sl(c, n0, nw),
                            start=(c == 0), stop=(c == DC - 1),
                        )
                osb = o_pool.tile([P, D], DT_IN, tag="osb", name="osb")
                # split the copy across both psum-capable engines
                nc.scalar.copy(osb[:, 0:384], po[:, 0:384])
                nc.vector.tensor_copy(osb[:, 384:D], po[:, 384:D])
                if i % 2 == 0:
                    nc.sync.dma_start(o_d[:, i * D:(i + 1) * D], osb[:])
                else:
                    nc.scalar.dma_start(o_d[:, i * D:(i + 1) * D], osb[:])

        wv_pool.release()
        persist.release()


# ---------------------------------------------------------------------------
# host side

def _build_masks():
    """masks[r] : [128, 17*128] bf16 = [identity | m0 | ... | m15]. Block j
    is applied (via an I-stationary matmul) to the first 128 q-cols of the
    ST_j window (global q slot j//2, global q block g = 2*(j//2) + r).

    g == j  -> causal tril (allow k <= q)
    g <  j  -> pad slot: kill the whole block
    g >  j  -> fully allowed
    """
    tril = np.tril(np.full((P, P), MASK_VAL, np.float32), -1)  # kill k > q
    out = []
    for r in (0, 1):
        m = np.zeros((SB, P, P), np.float32)
        for j in range(SB):
            g = 2 * (j // 2) + r
            if g == j:
                m[j] = tril
            elif g < j:
                m[j, :, :] = MASK_VAL
        # [j, p, col] -> [p, (1+j)*128+col], identity at cols [0:128]
        import ml_dtypes
        flat = np.concatenate(
            [np.eye(P, dtype=np.float32),
             m.transpose(1, 0, 2).reshape(P, SB * P)], axis=1)
        out.append(np.ascontiguousarray(flat.astype(ml_dtypes.bfloat16)))
    return out


def _perm_rows(a, nch):
    """[nch*128, W] -> [128, nch*W] with chunk c at cols [c*W:(c+1)*W]."""
    w = a.shape[1]
    return np.ascontiguousarray(
        a.reshape(nch, P, w).transpose(1, 0, 2).reshape(P, nch * w))


_STATE = {}


def _get_nc():
    if "nc" not in _STATE:
        nc = build_nc()
        nc.finalize()
        _STATE["nc"] = nc
    return _STATE["nc"]


def _cast_in(a):
    import ml_dtypes
    return np.ascontiguousarray(a.astype(ml_dtypes.bfloat16))


def build_in_maps(x, Wq, Wk, Wv):
    masks = _build_masks()
    G = np.ascontiguousarray(Wq.astype(np.float32) @ Wk.astype(np.float32).T)
    # co-major pack: row block co holds G[ci*128:(ci+1)*128, co*...] tiles.
    gp = np.concatenate(
        [np.concatenate([G[ci * P:(ci + 1) * P, co * P:(co + 1) * P]
                         for ci in range(DC)], axis=1) for co in range(DC)],
        axis=0)
    gqp = _cast_in(_perm_rows(gp, DC))                       # [128, 6*768]
    wvp = _cast_in(_perm_rows(np.asarray(Wv, np.float32), DC))
    in_maps = []
    for b in range(B):
        xb = np.asarray(x[b], np.float32)
        xt = xb.T                                            # [768, 2048]
        # xt j-major: [p, (j*6+c)*128+col] = XT[c*128+p, j*128+col]
        xtp = _cast_in(
            xt.reshape(DC, P, SB, P).transpose(1, 2, 0, 3).reshape(P, -1))
        xvp = _cast_in(_perm_rows(xb, SB))                   # [128, 16*768]
        for r in (0, 1):
            cols = np.concatenate(
                [xt[:, (2 * i + r) * P:(2 * i + r + 1) * P] for i in range(NQ)],
                axis=1,
            )                                                # [768, 1024]
            # xtq: [p, (g*6+c)*512+col] = cols[c*128+p, g*512+col]
            xtqp = _cast_in(
                cols.reshape(DC, P, 2, 512).transpose(1, 2, 0, 3).reshape(P, -1))
            in_maps.append({
                "xt": xtp,
                "xtq": xtqp,
                "gq": gqp, "xv": xvp, "wv": wvp,
                "mask": masks[r],
            })
    return in_maps


def kernel(x, Wq, Wk, Wv):
    x = np.ascontiguousarray(np.asarray(x, np.float32))
    Wq = np.ascontiguousarray(np.asarray(Wq, np.float32))
    Wk = np.ascontiguousarray(np.asarray(Wk, np.float32))
    Wv = np.ascontiguousarray(np.asarray(Wv, np.float32))

    from concourse.bass_utils import run_bass_kernel_spmd

    nc = _get_nc()
    in_maps = build_in_maps(x, Wq, Wk, Wv)

    res = run_bass_kernel_spmd(nc, in_maps, core_ids=list(range(8)), trace=False)

    out = np.empty((B, S, D), np.float32)
    for b in range(B):
        for r in (0, 1):
            o = np.asarray(res.results[2 * b + r]["o"], np.float32)
            # [p, i*768+col] -> rows g=2i+r
            ob = o.reshape(P, NQ, D).transpose(1, 0, 2)      # [8, 128, 768]
            for i in range(NQ):
                g = 2 * i + r
                out[b, g * P:(g + 1) * P, :] = ob[i]
    return out


# ---------------------------------------------------------------------------
# benchmarking support (not used by the grading path)

def _make_executor(nc, n_cores=8):
    """Build a cached jitted SPMD callable (no donation, reusable buffers)."""
    import jax
    from jax.sharding import Mesh, PartitionSpec
    try:
        from jax.experimental.shard_map import shard_map
    except ImportError:
        from jax.shard_map import shard_map
    from concourse import bass2jax
    from concourse import mybir as mb

    bass2jax.install_neuronx_cc_hook()
    partition_name = nc.partition_id_tensor.name if nc.partition_id_tensor else None
    in_names, out_names, out_avals, zero_outs = [], [], [], []
    for alloc in nc.m.functions[0].allocations:
        if not isinstance(alloc, mb.MemoryLocationSet):
            continue
        name = alloc.memorylocations[0].name
        if alloc.kind == "ExternalInput":
            if name != partition_name:
                in_names.append(name)
        elif alloc.kind == "ExternalOutput":
            shape = tuple(alloc.tensor_shape)
            dtype = mb.dt.np(alloc.dtype)
            out_names.append(name)
            out_avals.append(jax.core.ShapedArray(shape, dtype))
            zero_outs.append(np.zeros(shape, dtype))
    n_params = len(in_names)
    all_names = list(in_names) + list(out_names)
    if partition_name is not None:
        all_names.append(partition_name)

    def _body(*args):
        operands = list(args)
        if partition_name is not None:
            operands.append(bass2jax.partition_id_tensor())
        outs = bass2jax._bass_exec_p.bind(
            *operands,
            out_avals=tuple(out_avals),
            in_names=tuple(all_names),
            out_names=tuple(out_names),
            lowering_input_output_aliases=(),
            sim_require_finite=True,
            sim_require_nnan=True,
            nc=nc,
        )
        return tuple(outs)

    devices = jax.devices()[:n_cores]
    mesh = Mesh(np.asarray(devices), ("core",))
    in_specs = (PartitionSpec("core"),) * (n_params + len(out_names))
    out_specs = (PartitionSpec("core"),) * len(out_names)
    sharded = jax.jit(
        shard_map(_body, mesh=mesh, in_specs=in_specs, out_specs=out_specs,
                  check_rep=False),
        keep_unused=True,
    )
    return sharded, in_names, out_names, out_avals, zero_outs


def benchmark(in_maps, iters=20, n_cores=8):
    """Run the compiled kernel `iters` times back-to-back on device-resident
    inputs; returns (per_iter_seconds, results_core0_dict)."""
    import time as _time
    import jax

    nc = _get_nc()
    key = "exec"
    if key not in _STATE:
        _STATE[key] = _make_executor(nc, n_cores)
    sharded, in_names, out_names, out_avals, zero_outs = _STATE[key]

    concat_in = [
        np.concatenate([np.asarray(in_maps[c][n]) for c in range(n_cores)], axis=0)
        for n in in_names
    ]
    concat_zeros = [
        np.zeros((n_cores * z.shape[0], *z.shape[1:]), z.dtype) for z in zero_outs
    ]
    args = [jax.device_put(a) for a in concat_in + concat_zeros]
    jax.block_until_ready(args)

    outs = sharded(*args)          # warm-up / compile
    jax.block_until_ready(outs)
    t0 = _time.time()
    for _ in range(iters):
        outs = sharded(*args)
    jax.block_until_ready(outs)
    per_iter = (_time.time() - t0) / iters
    res0 = {
        n: np.asarray(outs[i]).reshape(n_cores, *out_avals[i].shape)[0]
        for i, n in enumerate(out_names)
    }
    return per_iter, res0


def measure_exec_ns(iters=12, reps_pair=(64, 96)):
    """Estimate true per-core HW execution time of one kernel body.

    Per-call wall time through the axon tunnel is dominated by a transfer
    floor proportional to I/O bytes (~16 ms) that completely hides execution.
    So we build NEFFs with the body repeated r1/r2 times (same I/O footprint)
    and use the slope: (wall(r2) - wall(r1)) / (r2 - r1).
    """
    import time as _time
    import jax

    rng = np.random.default_rng(0)
    x = rng.standard_normal((B, S, D)).astype(np.float32)
    sc = 1.0 / np.sqrt(D)
    Wq = rng.uniform(-sc, sc, (D, D)).astype(np.float32)
    Wk = rng.uniform(-sc, sc, (D, D)).astype(np.float32)
    Wv = rng.uniform(-sc, sc, (D, D)).astype(np.float32)
    in_maps = build_in_maps(x, Wq, Wk, Wv)

    pers = {}
    for reps in reps_pair:
        nc = build_nc(reps=reps)
        nc.finalize()
        sharded, in_names, out_names, out_avals, zero_outs = _make_executor(nc, 8)
        concat_in = [
            np.concatenate([np.asarray(in_maps[c][n]) for c in range(8)], axis=0)
            for n in in_names
        ]
        concat_zeros = [
            np.zeros((8 * z.shape[0], *z.shape[1:]), z.dtype) for z in zero_outs
        ]
        args = [jax.device_put(a) for a in concat_in + concat_zeros]
        jax.block_until_ready(args)
        outs = sharded(*args)
        jax.block_until_ready(outs)
        best = None
        for _ in range(3):
            t0 = _time.time()
            for _ in range(iters):
                outs = sharded(*args)
            jax.block_until_ready(outs)
            per = (_time.time() - t0) / iters
            best = per if best is None else min(best, per)
        pers[reps] = best
    r1, r2 = reps_pair
    return int((pers[r2] - pers[r1]) / (r2 - r1) * 1e9)
